# revision 128
# baseline (speedup 1.0000x reference)
"""BinaryLinear on 8 TRN2 NeuronCores.

Computes y = sign(x) @ sign(W)^T + bias for x [8192, 4096] fp32,
W [4096, 4096] fp32, bias [4096] fp32 -> y [8192, 4096] fp32.

Sharding: 4 batch shards x 2 contraction (K) shards. Core (bi, kh)
computes the transposed partial out^T_kh [4096 o, 2048 b] over its half
of the contraction dim; the kh=0 core also adds the bias (kh=1 cores
receive zeros). The host sums the two fp16 partials per batch shard
(each partial is an integer in [-2048, 2048], exact in fp16) and
transposes back. This K-split minimizes per-core HBM traffic:
16.8 MB x + 33.6 MB W + 16.8 MB fp16 out = 186.4 us at the simulated
360 GB/s, vs 210.6 us for the out-feature-sharded variant.

Because sign values are exactly +-1 (representable in fp8e4) and the PE
accumulates in fp32, the fp8 DoubleRow matmul result is bit-exact.

Per-core kernel (Tile framework):
  - Pairing layout for fp8 DoubleRow (pair index h):
      k = 256*t + 2*p + h  at PE partition p, pair-group t.
  - W: fp32 half-row tiles -> Sign on ScalarE (SBUF->SBUF fp8) -> PE
    transposes as 2-byte pairs -> reversed-column VectorE copies -> wT
    in DoubleRowSwInterleave layout [128, T, 4096] (u16 pairs).
  - x: fp32 half-row tiles -> Sign on ScalarE (SBUF->SBUF fp8) -> PE
    transposes of the fp8 data viewed as 2-byte pairs -> VectorE copies
    PSUM->SBUF -> xTp [128, T, 512] per batch chunk.
  - TensorE fp8 DoubleRowSwInterleave matmuls (256 k per pass), fp32
    PSUM accumulation.
  - VectorE bias add (per-partition scalar) converting to fp16 output.
  - Software-pipelined slot schedule (one slot per W row tile) with
    dedicated queues so no stream head-of-line blocks another:
    SP ring = ALL DMA issues (W loads, x loads, stores; stores lag
    their matmul by 3 slots so they never park the queue); ACT ring =
    signs only; PE = transposes + matmuls, with x transposes two slots
    and matmuls one slot behind their producers so the PE queue never
    waits mid-stream (the sim's PE p-state model roughly doubles matmul
    cost after every PE idle). DMA runs gap-free from first load to
    last store.
"""

from contextlib import ExitStack

import numpy as np

import concourse.bacc as bacc
import concourse.bass as bass
import concourse.mybir as mybir
import concourse.tile as tile
from concourse import masks
from concourse.bass_utils import run_bass_kernel_spmd

F32 = mybir.dt.float32
AF = mybir.ActivationFunctionType
ALU = mybir.AluOpType

B, IN, OUT = 8192, 4096, 4096
P_B, P_K = 4, 2                  # batch shards x contraction (K) shards
Bs, Os, Ks = B // P_B, OUT, IN // P_K   # per-core shard sizes
N_CORES = P_B * P_K


def _build(Bs=2048, Os=4096, K=2048, BC=512, mm_dtype=mybir.dt.float8e4):
    nc = bacc.Bacc("TRN2", target_bir_lowering=False, debug=False)

    x = nc.declare_dram_parameter("x", [Bs, K], F32, isOutput=False)
    w = nc.declare_dram_parameter("w", [Os, K], F32, isOutput=False)
    b = nc.declare_dram_parameter("b", [Os], F32, isOutput=False)
    # fp16 output: each K-half partial is an integer in [-K, K] (exact in
    # fp16); the kh=0 core also adds the fp32 bias (kh=1 cores get zeros),
    # so the host unshard is a plain sum of the two partials.
    out = nc.declare_dram_parameter("out", [Os, Bs], mybir.dt.float16, isOutput=True)

    T = K // 256       # DoubleRow pair-groups
    OT = Os // 128     # output row tiles (partition dim of out^T)
    NB = Bs // BC      # batch chunks
    SB = BC // 128     # 128-row sub-tiles per batch chunk
    KH = K // 2        # W half-tile columns
    TH = T // 2        # pair-groups per W half-tile

    with tile.TileContext(nc) as tc, ExitStack() as ctx:
        const = ctx.enter_context(tc.tile_pool(name="const", bufs=1))
        wt_pool = ctx.enter_context(tc.tile_pool(name="wt", bufs=1))
        xt_pool = ctx.enter_context(tc.tile_pool(name="xt", bufs=4))
        xstage = ctx.enter_context(tc.tile_pool(name="xstage", bufs=4))
        sx_pool = ctx.enter_context(tc.tile_pool(name="sx", bufs=2))
        sw_pool = ctx.enter_context(tc.tile_pool(name="sw", bufs=4))
        wstage = ctx.enter_context(tc.tile_pool(name="wstage", bufs=5))
        ptrx_pool = ctx.enter_context(
            tc.tile_pool(name="ptrx", bufs=2, space=bass.MemorySpace.PSUM)
        )
        ptrw_pool = ctx.enter_context(
            tc.tile_pool(name="ptrw", bufs=2, space=bass.MemorySpace.PSUM)
        )
        pacc_pool = ctx.enter_context(
            tc.tile_pool(name="pacc", bufs=4, space=bass.MemorySpace.PSUM)
        )
        outsb = ctx.enter_context(tc.tile_pool(name="outsb", bufs=16))

        ident16 = const.tile([128, 128], mybir.dt.bfloat16)
        masks.make_identity(nc, ident16[:])



        bias_sb = const.tile([128, OT], F32)
        nc.sync.dma_start(bias_sb[:], b.rearrange("(ot p) -> p ot", p=128))

        # wT u16-pair view [128, T, Os]: partition p of group t holds the
        # fp8 byte pair (k = 256t+2p, 256t+2p+1); m reversed per ot block.
        wT = wt_pool.tile([128, T, Os], mybir.dt.bfloat16)

        def load_w_tile(ot):
            """[SP] load W rows [ot*128, +128) in two column halves."""
            halves = []
            for hh in range(2):
                ws = wstage.tile([128, KH], F32, tag="ws", name="ws")
                nc.sync.dma_start(
                    ws[:], w[ot * 128:(ot + 1) * 128, hh * KH:(hh + 1) * KH]
                )
                halves.append(ws)
            return halves

        def sign_w_tile(halves):
            """[ACT] sw = sign(ws) as fp8 +-1."""
            sws = []
            for ws in halves:
                sw = sw_pool.tile([128, KH], mm_dtype, tag="sw", name="sw")
                nc.scalar.activation(sw[:], ws[:], AF.Sign)
                sws.append(sw)
            return sws

        def trans_w_tile(ot, sws):
            """[PE+DVE] u16-pair transposes -> reversed-m copies into wT."""
            for hh in range(2):
                sw16 = sws[hh].bitcast(mybir.dt.bfloat16)   # [128, KH//2]
                GW = min(8, TH)
                for tg in range(TH // GW):
                    ptx = ptrw_pool.tile(
                        [128, GW * 128], mybir.dt.bfloat16,
                        tag="ptrw", name="ptxw",
                    )
                    for j in range(GW):
                        lt = tg * GW + j
                        nc.tensor.transpose(
                            ptx[:, j * 128:(j + 1) * 128],
                            sw16[:, lt * 128:(lt + 1) * 128],
                            ident16[:],
                        )
                    t0 = hh * TH + tg * GW
                    # reversed-m copy within this ot block (SwInterleave)
                    nc.vector.tensor_copy(
                        wT[:, t0:t0 + GW, ot * 128:(ot + 1) * 128][:, :, ::-1],
                        ptx[:],
                    )

        def load_x_tile(c, s):
            """[ACT] load one 128-row x sub-tile in two column halves."""
            halves = []
            for hh in range(2):
                xs = xstage.tile([128, KH], F32, tag="xs", name="xs")
                nc.sync.dma_start(
                    xs[:],
                    x[(c * SB + s) * 128:(c * SB + s + 1) * 128,
                      hh * KH:(hh + 1) * KH],
                )
                halves.append(xs)
            return halves

        def sign_x_half_act(halves):
            """[ACT] sign of column half 0 -> fresh sx tile."""
            sx = sx_pool.tile([128, K], mm_dtype, tag="sx", name="sx")
            nc.scalar.activation(sx[:, 0:KH], halves[0][:], AF.Sign)
            return sx

        def sign_x_half_b(halves, sx):
            """[ACT] sign of column half 1 into sx."""
            nc.scalar.activation(sx[:, KH:K], halves[1][:], AF.Sign)

        def alloc_xtp():
            # bf16-as-u16 view: [128, T, BC]; partition p of group t holds
            # the fp8 byte pair (k = 256t+2p, 256t+2p+1) per batch col.
            return xt_pool.tile([128, T, BC], mybir.dt.bfloat16, name="xTp", tag="xTp")

        def transpose_x_subtile(sx, xTp, s):
            """PE-transpose the signed sub-tile as 2-byte pairs into xTp.

            sx fp8 [128 b, K] viewed as bf16 [128, K//2]; each [128,128]
            block t transposes to [128 pair-idx, 128 b] in PSUM, then DVE
            copies 8 blocks per bank into xTp[:, t0:t0+8, s*128:+128].
            """
            sx16 = sx.bitcast(mybir.dt.bfloat16)   # [128, K//2]
            GX = min(8, T)
            for tg in range(T // GX):
                ptx = ptrx_pool.tile([128, GX * 128], mybir.dt.bfloat16,
                                     tag="ptrx", name="ptrx")
                for j in range(GX):
                    t = tg * GX + j
                    nc.tensor.transpose(
                        ptx[:, j * 128:(j + 1) * 128],
                        sx16[:, t * 128:(t + 1) * 128],
                        ident16[:],
                    )
                nc.vector.tensor_copy(
                    xTp[:, tg * GX:(tg + 1) * GX, s * 128:(s + 1) * 128],
                    ptx[:],
                )

        def mm_tile(ot, bc, xTp):
            """DoubleRow matmuls for one out^T tile + bias->fp16 in SBUF."""
            xTp8 = xTp.bitcast(mm_dtype)   # [128, T, 2*BC]
            pacc = pacc_pool.tile([128, BC], F32, name="pacc", tag="pacc")
            for t in range(T):
                rhs = xTp8[:, t, :].rearrange("p (b h) -> p h b", h=2)
                nc.tensor.matmul(
                    pacc[:],
                    wT[:, t, ot * 128:(ot + 1) * 128].bitcast(mm_dtype),
                    rhs,
                    start=(t == 0),
                    stop=(t == T - 1),
                    perf_mode=mybir.MatmulPerfMode.DoubleRowSwInterleave,
                )
            osb = outsb.tile([128, BC], mybir.dt.float16, name="osb", tag="osb")
            nc.vector.tensor_scalar(
                osb[:], pacc[:], bias_sb[:, ot:ot + 1], None, ALU.add
            )
            return osb

        def store_tile(ot, bc, osb):
            """[SP] store one out^T tile (emitted with a 3-slot lag, so
            its wait can never park the load streams)."""
            nc.sync.dma_start(
                out[ot * 128:(ot + 1) * 128, bc * BC:(bc + 1) * BC], osb[:]
            )

        # ---- software-pipelined slot schedule (one slot per W row-tile) --
        # Queues: SP = W loads + out stores; ACT = x loads + x signs;
        # Pool = W signs; PE = transposes + matmuls; DVE = copies + bias.
        # Stage lags (slot s): load W[s+2], sign W[s+1], transpose W[s],
        # mm units with wT[ot<=s]; x loads pace XPACE subtiles/slot with
        # sign/transpose one slot behind; stores one slot behind their mm.
        xs_tiles = {}
        sx_tiles = {}
        xTp = {}

        rest = [(c, s) for c in range(1, NB) for s in range(SB)]
        # tapered x pace: flood the first slots while ACT is free, then
        # ease off so the x-sign backlog never delays W signs on ACT
        xslots = [0, 0, 0, 0, 0, 1, 1, 1, 1, 2, 2, 2]
        loads_at, signs_at, trs_at = {}, {}, {}
        ready_slot = {0: 1}
        for i, cs in enumerate(rest):
            slot = xslots[i] if i < len(xslots) else 3 + (i - len(xslots))
            loads_at.setdefault(slot, []).append(cs)
            signs_at.setdefault(slot + 1, []).append(cs)
            trs_at.setdefault(slot + 2, []).append(cs)
            ready_slot[cs[0]] = max(ready_slot.get(cs[0], 0), slot + 3)

        # W pipeline stage slots. Steady state: load tile t at slot t-2,
        # sign at t-1, transpose at t, units from t+1 (so PE never bubbles
        # on the transpose->DVE-copy handoff).
        load_slot = {t: t - 2 for t in range(2, OT)}
        sign_slot = {t: t - 1 for t in range(1, OT)}
        trans_slot = {t: t for t in range(OT)}
        loads_w_at, signs_w_at, trans_w_at = {}, {}, {}
        for t, s in load_slot.items():
            loads_w_at.setdefault(s, []).append(t)
        for t, s in sign_slot.items():
            signs_w_at.setdefault(s, []).append(t)
        for t, s in trans_slot.items():
            trans_w_at.setdefault(s, []).append(t)

        NSLOT = OT + 1
        avail = sorted(
            (max(trans_slot[ot] + (0 if ot >= OT - 2 else 1), ready_slot[c]), ot, c)
            for c in range(NB)
            for ot in range(OT)
        )
        mms_at = {}
        taken = 0
        for slot in range(NSLOT):
            rem = len(avail) - taken
            cap = rem if slot == NSLOT - 1 else -(-rem // (NSLOT - slot))
            if slot < NSLOT - 3:
                cap += 1
            picked = []
            while len(picked) < cap and taken < len(avail):
                ready, ot, c = avail[taken]
                if ready > slot:
                    break
                picked.append((ot, c))
                taken += 1
            mms_at[slot] = picked

        # head: stage W tiles 0-1 and chunk 0 ahead of the steady loop
        w_halves = {0: load_w_tile(0), 1: load_w_tile(1)}
        w_signed = {0: sign_w_tile(w_halves.pop(0))}
        xTp[0] = alloc_xtp()
        for s in range(SB):
            xs_tiles[(0, s)] = load_x_tile(0, s)
        for s in range(SB):
            sx_tiles[(0, s)] = sign_x_half_act(xs_tiles[(0, s)])
            sign_x_half_b(xs_tiles[(0, s)], sx_tiles[(0, s)])

        store_q = {}
        for slot in range(NSLOT):
            # [SP] W loads scheduled this slot
            for t in loads_w_at.get(slot, []):
                w_halves[t] = load_w_tile(t)
            # [ACT] W signs scheduled this slot
            for t in signs_w_at.get(slot, []):
                w_signed[t] = sign_w_tile(w_halves.pop(t))
            # [ACT] signs of last slot's x loads, then new x loads
            for c, s in signs_at.get(slot, []):
                sx_tiles[(c, s)] = sign_x_half_act(xs_tiles[(c, s)])
                sign_x_half_b(xs_tiles[(c, s)], sx_tiles[(c, s)])
            for c, s in loads_at.get(slot, []):
                xs_tiles[(c, s)] = load_x_tile(c, s)
            # [PE+DVE] wT transposes/copies scheduled this slot
            for t in trans_w_at.get(slot, []):
                trans_w_tile(t, w_signed.pop(t))
            # [PE] chunk-0 transposes in slot 0 (signed in the head)
            if slot == 0:
                for s in range(SB):
                    transpose_x_subtile(sx_tiles[(0, s)], xTp[0], s)
            # [PE] matmul units ready this slot
            stores = []
            for ot, c in mms_at.get(slot, []):
                stores.append((ot, c, mm_tile(ot, c, xTp[c])))
            store_q[slot] = stores
            # [PE] x transposes of sub-tiles signed earlier this slot
            for c, s in trs_at.get(slot, []):
                if c not in xTp:
                    xTp[c] = alloc_xtp()
                transpose_x_subtile(sx_tiles[(c, s)], xTp[c], s)
            # [ACT] stores from two slots ago, after this slot's signs so
            # store issue time never delays wstage/xstage recycling
            for ot, c, osb in store_q.pop(slot - 3, []):
                store_tile(ot, c, osb)
        for sl in sorted(store_q):
            for ot, c, osb in store_q[sl]:
                store_tile(ot, c, osb)

    nc.compile()
    return nc


_NC_CACHE = None


def kernel(x: np.ndarray, weight: np.ndarray, bias: np.ndarray) -> np.ndarray:
    global _NC_CACHE
    if _NC_CACHE is None:
        _NC_CACHE = _build()
    nc = _NC_CACHE

    x = np.ascontiguousarray(np.asarray(x, dtype=np.float32))
    weight = np.ascontiguousarray(np.asarray(weight, dtype=np.float32))
    bias = np.ascontiguousarray(np.asarray(bias, dtype=np.float32))

    zeros = np.zeros_like(bias)
    in_maps = []
    for c in range(N_CORES):
        bi, kh = c // P_K, c % P_K
        in_maps.append(
            {
                "x": np.ascontiguousarray(
                    x[bi * Bs:(bi + 1) * Bs, kh * Ks:(kh + 1) * Ks]
                ),
                "w": np.ascontiguousarray(weight[:, kh * Ks:(kh + 1) * Ks]),
                "b": bias if kh == 0 else zeros,
            }
        )

    res = run_bass_kernel_spmd(nc, in_maps, list(range(N_CORES)))

    out = np.empty((B, OUT), dtype=np.float32)
    for bi in range(P_B):
        p0 = res.results[bi * P_K]["out"].astype(np.float32)
        p1 = res.results[bi * P_K + 1]["out"].astype(np.float32)
        out[bi * Bs:(bi + 1) * Bs, :] = (p0 + p1).T
    return out



# revision 130
# speedup vs baseline: 1.0007x; 1.0007x over previous
"""BinaryLinear on 8 TRN2 NeuronCores.

Computes y = sign(x) @ sign(W)^T + bias for x [8192, 4096] fp32,
W [4096, 4096] fp32, bias [4096] fp32 -> y [8192, 4096] fp32.

Sharding: 4 batch shards x 2 contraction (K) shards. Core (bi, kh)
computes the transposed partial out^T_kh [4096 o, 2048 b] over its half
of the contraction dim; the kh=0 core also adds the bias (kh=1 cores
receive zeros). The host sums the two fp16 partials per batch shard
(each partial is an integer in [-2048, 2048], exact in fp16) and
transposes back. This K-split minimizes per-core HBM traffic:
16.8 MB x + 33.6 MB W + 16.8 MB fp16 out = 186.4 us at the simulated
360 GB/s, vs 210.6 us for the out-feature-sharded variant.

Because sign values are exactly +-1 (representable in fp8e4) and the PE
accumulates in fp32, the fp8 DoubleRow matmul result is bit-exact.

Per-core kernel (Tile framework):
  - Pairing layout for fp8 DoubleRow (pair index h):
      k = 256*t + 2*p + h  at PE partition p, pair-group t.
  - W: fp32 half-row tiles -> Sign on ScalarE (SBUF->SBUF fp8) -> PE
    transposes as 2-byte pairs -> reversed-column VectorE copies -> wT
    in DoubleRowSwInterleave layout [128, T, 4096] (u16 pairs).
  - x: fp32 half-row tiles -> Sign on ScalarE (SBUF->SBUF fp8) -> PE
    transposes of the fp8 data viewed as 2-byte pairs -> VectorE copies
    PSUM->SBUF -> xTp [128, T, 512] per batch chunk.
  - TensorE fp8 DoubleRowSwInterleave matmuls (256 k per pass), fp32
    PSUM accumulation.
  - VectorE bias add (per-partition scalar) converting to fp16 output.
  - Software-pipelined slot schedule (one slot per W row tile) with
    dedicated queues so no stream head-of-line blocks another:
    SP ring = ALL DMA issues (W loads, x loads, stores; stores lag
    their matmul by 3 slots so they never park the queue); ACT ring =
    signs only; PE = transposes + matmuls, with x transposes two slots
    and matmuls one slot behind their producers so the PE queue never
    waits mid-stream (the sim's PE p-state model roughly doubles matmul
    cost after every PE idle). DMA runs gap-free from first load to
    last store.
"""

from contextlib import ExitStack

import numpy as np

import concourse.bacc as bacc
import concourse.bass as bass
import concourse.mybir as mybir
import concourse.tile as tile
from concourse import masks
from concourse.bass_utils import run_bass_kernel_spmd

F32 = mybir.dt.float32
AF = mybir.ActivationFunctionType
ALU = mybir.AluOpType

B, IN, OUT = 8192, 4096, 4096
P_B, P_K = 4, 2                  # batch shards x contraction (K) shards
Bs, Os, Ks = B // P_B, OUT, IN // P_K   # per-core shard sizes
N_CORES = P_B * P_K


def _build(Bs=2048, Os=4096, K=2048, BC=512, mm_dtype=mybir.dt.float8e4):
    nc = bacc.Bacc("TRN2", target_bir_lowering=False, debug=False)

    x = nc.declare_dram_parameter("x", [Bs, K], F32, isOutput=False)
    w = nc.declare_dram_parameter("w", [Os, K], F32, isOutput=False)
    b = nc.declare_dram_parameter("b", [Os], F32, isOutput=False)
    # fp16 output: each K-half partial is an integer in [-K, K] (exact in
    # fp16); the kh=0 core also adds the fp32 bias (kh=1 cores get zeros),
    # so the host unshard is a plain sum of the two partials.
    out = nc.declare_dram_parameter("out", [Os, Bs], mybir.dt.float16, isOutput=True)

    T = K // 256       # DoubleRow pair-groups
    OT = Os // 128     # output row tiles (partition dim of out^T)
    NB = Bs // BC      # batch chunks
    SB = BC // 128     # 128-row sub-tiles per batch chunk
    KH = K // 2        # W half-tile columns
    TH = T // 2        # pair-groups per W half-tile

    with tile.TileContext(nc) as tc, ExitStack() as ctx:
        const = ctx.enter_context(tc.tile_pool(name="const", bufs=1))
        wt_pool = ctx.enter_context(tc.tile_pool(name="wt", bufs=1))
        xt_pool = ctx.enter_context(tc.tile_pool(name="xt", bufs=4))
        xstage = ctx.enter_context(tc.tile_pool(name="xstage", bufs=4))
        sx_pool = ctx.enter_context(tc.tile_pool(name="sx", bufs=2))
        sw_pool = ctx.enter_context(tc.tile_pool(name="sw", bufs=4))
        wstage = ctx.enter_context(tc.tile_pool(name="wstage", bufs=5))
        ptrx_pool = ctx.enter_context(
            tc.tile_pool(name="ptrx", bufs=2, space=bass.MemorySpace.PSUM)
        )
        ptrw_pool = ctx.enter_context(
            tc.tile_pool(name="ptrw", bufs=2, space=bass.MemorySpace.PSUM)
        )
        pacc_pool = ctx.enter_context(
            tc.tile_pool(name="pacc", bufs=4, space=bass.MemorySpace.PSUM)
        )
        outsb = ctx.enter_context(tc.tile_pool(name="outsb", bufs=16))

        ident16 = const.tile([128, 128], mybir.dt.bfloat16)
        masks.make_identity(nc, ident16[:])



        bias_sb = const.tile([128, OT], F32)
        nc.sync.dma_start(bias_sb[:], b.rearrange("(ot p) -> p ot", p=128))

        # wT u16-pair view [128, T, Os]: partition p of group t holds the
        # fp8 byte pair (k = 256t+2p, 256t+2p+1); m reversed per ot block.
        wT = wt_pool.tile([128, T, Os], mybir.dt.bfloat16)

        def load_w_tile(ot):
            """[SP] load W rows [ot*128, +128) in two column halves."""
            halves = []
            for hh in range(2):
                ws = wstage.tile([128, KH], F32, tag="ws", name="ws")
                nc.sync.dma_start(
                    ws[:], w[ot * 128:(ot + 1) * 128, hh * KH:(hh + 1) * KH]
                )
                halves.append(ws)
            return halves

        def sign_w_tile(halves):
            """[ACT] sw = sign(ws) as fp8 +-1."""
            sws = []
            for ws in halves:
                sw = sw_pool.tile([128, KH], mm_dtype, tag="sw", name="sw")
                nc.scalar.activation(sw[:], ws[:], AF.Sign)
                sws.append(sw)
            return sws

        def trans_w_tile(ot, sws):
            """[PE+DVE] u16-pair transposes -> reversed-m copies into wT."""
            for hh in range(2):
                sw16 = sws[hh].bitcast(mybir.dt.bfloat16)   # [128, KH//2]
                GW = min(8, TH)
                for tg in range(TH // GW):
                    ptx = ptrw_pool.tile(
                        [128, GW * 128], mybir.dt.bfloat16,
                        tag="ptrw", name="ptxw",
                    )
                    for j in range(GW):
                        lt = tg * GW + j
                        nc.tensor.transpose(
                            ptx[:, j * 128:(j + 1) * 128],
                            sw16[:, lt * 128:(lt + 1) * 128],
                            ident16[:],
                        )
                    t0 = hh * TH + tg * GW
                    # reversed-m copy within this ot block (SwInterleave)
                    nc.vector.tensor_copy(
                        wT[:, t0:t0 + GW, ot * 128:(ot + 1) * 128][:, :, ::-1],
                        ptx[:],
                    )

        def load_x_tile(c, s):
            """[ACT] load one 128-row x sub-tile in two column halves."""
            halves = []
            for hh in range(2):
                xs = xstage.tile([128, KH], F32, tag="xs", name="xs")
                nc.sync.dma_start(
                    xs[:],
                    x[(c * SB + s) * 128:(c * SB + s + 1) * 128,
                      hh * KH:(hh + 1) * KH],
                )
                halves.append(xs)
            return halves

        def sign_x_half_act(halves):
            """[ACT] sign of column half 0 -> fresh sx tile."""
            sx = sx_pool.tile([128, K], mm_dtype, tag="sx", name="sx")
            nc.scalar.activation(sx[:, 0:KH], halves[0][:], AF.Sign)
            return sx

        def sign_x_half_b(halves, sx):
            """[ACT] sign of column half 1 into sx."""
            nc.scalar.activation(sx[:, KH:K], halves[1][:], AF.Sign)

        def alloc_xtp():
            # bf16-as-u16 view: [128, T, BC]; partition p of group t holds
            # the fp8 byte pair (k = 256t+2p, 256t+2p+1) per batch col.
            return xt_pool.tile([128, T, BC], mybir.dt.bfloat16, name="xTp", tag="xTp")

        def transpose_x_subtile(sx, xTp, s):
            """PE-transpose the signed sub-tile as 2-byte pairs into xTp.

            sx fp8 [128 b, K] viewed as bf16 [128, K//2]; each [128,128]
            block t transposes to [128 pair-idx, 128 b] in PSUM, then DVE
            copies 8 blocks per bank into xTp[:, t0:t0+8, s*128:+128].
            """
            sx16 = sx.bitcast(mybir.dt.bfloat16)   # [128, K//2]
            GX = min(8, T)
            for tg in range(T // GX):
                ptx = ptrx_pool.tile([128, GX * 128], mybir.dt.bfloat16,
                                     tag="ptrx", name="ptrx")
                for j in range(GX):
                    t = tg * GX + j
                    nc.tensor.transpose(
                        ptx[:, j * 128:(j + 1) * 128],
                        sx16[:, t * 128:(t + 1) * 128],
                        ident16[:],
                    )
                nc.vector.tensor_copy(
                    xTp[:, tg * GX:(tg + 1) * GX, s * 128:(s + 1) * 128],
                    ptx[:],
                )

        def mm_tile(ot, bc, xTp):
            """DoubleRow matmuls for one out^T tile + bias->fp16 in SBUF."""
            xTp8 = xTp.bitcast(mm_dtype)   # [128, T, 2*BC]
            pacc = pacc_pool.tile([128, BC], F32, name="pacc", tag="pacc")
            for t in range(T):
                rhs = xTp8[:, t, :].rearrange("p (b h) -> p h b", h=2)
                nc.tensor.matmul(
                    pacc[:],
                    wT[:, t, ot * 128:(ot + 1) * 128].bitcast(mm_dtype),
                    rhs,
                    start=(t == 0),
                    stop=(t == T - 1),
                    perf_mode=mybir.MatmulPerfMode.DoubleRowSwInterleave,
                )
            osb = outsb.tile([128, BC], mybir.dt.float16, name="osb", tag="osb")
            nc.vector.tensor_scalar(
                osb[:], pacc[:], bias_sb[:, ot:ot + 1], None, ALU.add
            )
            return osb

        def store_tile(ot, bc, osb):
            """[SP] store one out^T tile (emitted with a 3-slot lag, so
            its wait can never park the load streams)."""
            nc.sync.dma_start(
                out[ot * 128:(ot + 1) * 128, bc * BC:(bc + 1) * BC], osb[:]
            )

        # ---- software-pipelined slot schedule (one slot per W row-tile) --
        # Queues: SP = W loads + out stores; ACT = x loads + x signs;
        # Pool = W signs; PE = transposes + matmuls; DVE = copies + bias.
        # Stage lags (slot s): load W[s+2], sign W[s+1], transpose W[s],
        # mm units with wT[ot<=s]; x loads pace XPACE subtiles/slot with
        # sign/transpose one slot behind; stores one slot behind their mm.
        xs_tiles = {}
        sx_tiles = {}
        xTp = {}

        rest = [(c, s) for c in range(1, NB) for s in range(SB)]
        # tapered x pace: flood the first slots while ACT is free, then
        # ease off so the x-sign backlog never delays W signs on ACT
        xslots = [0, 0, 0, 0, 0, 1, 1, 1, 1, 2, 2, 2]
        loads_at, signs_at, trs_at = {}, {}, {}
        ready_slot = {0: 1}
        for i, cs in enumerate(rest):
            slot = xslots[i] if i < len(xslots) else 3 + (i - len(xslots))
            loads_at.setdefault(slot, []).append(cs)
            signs_at.setdefault(slot + 1, []).append(cs)
            trs_at.setdefault(slot + 2, []).append(cs)
            ready_slot[cs[0]] = max(ready_slot.get(cs[0], 0), slot + 3)

        # W pipeline stage slots. Steady state: load tile t at slot t-2,
        # sign at t-1, transpose at t, units from t+1 (so PE never bubbles
        # on the transpose->DVE-copy handoff).
        load_slot = {t: t - 2 for t in range(2, OT)}
        sign_slot = {t: t - 1 for t in range(1, OT)}
        trans_slot = {t: t for t in range(OT)}
        loads_w_at, signs_w_at, trans_w_at = {}, {}, {}
        for t, s in load_slot.items():
            loads_w_at.setdefault(s, []).append(t)
        for t, s in sign_slot.items():
            signs_w_at.setdefault(s, []).append(t)
        for t, s in trans_slot.items():
            trans_w_at.setdefault(s, []).append(t)

        NSLOT = OT + 1
        avail = sorted(
            (max(trans_slot[ot] + 1, ready_slot[c]), ot, c)
            for c in range(NB)
            for ot in range(OT)
        )
        mms_at = {}
        taken = 0
        for slot in range(NSLOT):
            rem = len(avail) - taken
            cap = rem if slot == NSLOT - 1 else -(-rem // (NSLOT - slot))
            if slot < NSLOT - 3:
                cap += 1
            picked = []
            while len(picked) < cap and taken < len(avail):
                ready, ot, c = avail[taken]
                if ready > slot:
                    break
                picked.append((ot, c))
                taken += 1
            mms_at[slot] = picked

        # head: stage W tiles 0-1 and chunk 0 ahead of the steady loop
        w_halves = {0: load_w_tile(0), 1: load_w_tile(1)}
        w_signed = {0: sign_w_tile(w_halves.pop(0))}
        xTp[0] = alloc_xtp()
        for s in range(SB):
            xs_tiles[(0, s)] = load_x_tile(0, s)
        for s in range(SB):
            sx_tiles[(0, s)] = sign_x_half_act(xs_tiles[(0, s)])
            sign_x_half_b(xs_tiles[(0, s)], sx_tiles[(0, s)])

        store_q = {}
        for slot in range(NSLOT):
            # [SP] W loads scheduled this slot
            for t in loads_w_at.get(slot, []):
                w_halves[t] = load_w_tile(t)
            # [ACT] W signs scheduled this slot
            for t in signs_w_at.get(slot, []):
                w_signed[t] = sign_w_tile(w_halves.pop(t))
            # [ACT] signs of last slot's x loads, then new x loads
            for c, s in signs_at.get(slot, []):
                sx_tiles[(c, s)] = sign_x_half_act(xs_tiles[(c, s)])
                sign_x_half_b(xs_tiles[(c, s)], sx_tiles[(c, s)])
            for c, s in loads_at.get(slot, []):
                xs_tiles[(c, s)] = load_x_tile(c, s)
            # [PE+DVE] wT transposes/copies scheduled this slot
            for t in trans_w_at.get(slot, []):
                trans_w_tile(t, w_signed.pop(t))
            # [PE] chunk-0 transposes in slot 0 (signed in the head)
            if slot == 0:
                for s in range(SB):
                    transpose_x_subtile(sx_tiles[(0, s)], xTp[0], s)
            # [PE] matmul units ready this slot
            stores = []
            for ot, c in mms_at.get(slot, []):
                stores.append((ot, c, mm_tile(ot, c, xTp[c])))
            store_q[slot] = stores
            # [PE] x transposes of sub-tiles signed earlier this slot
            for c, s in trs_at.get(slot, []):
                if c not in xTp:
                    xTp[c] = alloc_xtp()
                transpose_x_subtile(sx_tiles[(c, s)], xTp[c], s)
            # [ACT] stores from two slots ago, after this slot's signs so
            # store issue time never delays wstage/xstage recycling
            for ot, c, osb in store_q.pop(slot - 3, []):
                store_tile(ot, c, osb)
        drain = [s for sl in sorted(store_q) for s in store_q[sl]]
        for i, (ot, c, osb) in enumerate(drain):
            # alternate queues so the final stores aren't SP-SEQ-paced
            if i % 2 == 0:
                store_tile(ot, c, osb)
            else:
                nc.scalar.dma_start(
                    out[ot * 128:(ot + 1) * 128, c * BC:(c + 1) * BC], osb[:]
                )

    nc.compile()
    return nc


_NC_CACHE = None


def kernel(x: np.ndarray, weight: np.ndarray, bias: np.ndarray) -> np.ndarray:
    global _NC_CACHE
    if _NC_CACHE is None:
        _NC_CACHE = _build()
    nc = _NC_CACHE

    x = np.ascontiguousarray(np.asarray(x, dtype=np.float32))
    weight = np.ascontiguousarray(np.asarray(weight, dtype=np.float32))
    bias = np.ascontiguousarray(np.asarray(bias, dtype=np.float32))

    zeros = np.zeros_like(bias)
    in_maps = []
    for c in range(N_CORES):
        bi, kh = c // P_K, c % P_K
        in_maps.append(
            {
                "x": np.ascontiguousarray(
                    x[bi * Bs:(bi + 1) * Bs, kh * Ks:(kh + 1) * Ks]
                ),
                "w": np.ascontiguousarray(weight[:, kh * Ks:(kh + 1) * Ks]),
                "b": bias if kh == 0 else zeros,
            }
        )

    res = run_bass_kernel_spmd(nc, in_maps, list(range(N_CORES)))

    out = np.empty((B, OUT), dtype=np.float32)
    for bi in range(P_B):
        p0 = res.results[bi * P_K]["out"].astype(np.float32)
        p1 = res.results[bi * P_K + 1]["out"].astype(np.float32)
        out[bi * Bs:(bi + 1) * Bs, :] = (p0 + p1).T
    return out

